# revision 1
# baseline (speedup 1.0000x reference)
"""Causal self-attention Trainium2 kernel (8 NeuronCores, SPMD).

Problem: B=4, T=2048, C=1024, H=16 heads, D=64.
  qkv = x @ w_attn + b_attn ; causal softmax attention ; out = y @ w_proj + b_proj

Sharding: core c = 2*b + g  handles batch b with head-group g (heads 8g..8g+7).
Each core computes a partial projection output (its 8 heads' contribution);
the host sums the two partials per batch and adds b_proj.

Per-core layouts (all fp32 in HBM, consumed as float32r by the PE):
  xT  [1024, 2048]  = x[b].T                       (contraction dim on partitions)
  w_q/w_k/w_v [1024, 512], b_q/b_k/b_v [512]       (head-group slices of w_attn)
  w_p [512, 1024]                                  (head-group rows of w_proj)
  mask [128, 896]   mask[tk, m] = 1 if m >= tk+384 (sliced per diagonal offset)

On-chip:
  qT, kT  [128, 4, 2048]  (d-group, t)  - head pair per 128 partitions
  v       [128, 16, 520]  (t-tile, 8 heads x (64 v | 1 ones))  ones col -> denominator
  sT tile [128 tk, 512 tq] = kT.T @ qT ; p = exp(s/8) (ScalarE, scale folded)
  y_psum  [65, 512] accumulates [v|1].T @ p over kt tiles; row 64 = softmax denom
  yT      [128, 4, 2048]  normalized via reciprocal + gpsimd partition_broadcast
  out     [2048, 1024] = yT.T @ w_p  (partial; host adds pair + b_proj)
"""

import sys
import os

sys.path.insert(0, "/opt/trn_rl_repo")

import numpy as np
import concourse.bass as bass
import concourse.mybir as mybir
import concourse.tile as tile
from concourse.vector_clock import ScopedClock
from concourse.bass_utils import run_bass_kernel_spmd

F32 = mybir.dt.float32
F32R = mybir.dt.float32r
EXP = mybir.ActivationFunctionType.Exp

B, T, C, H = 4, 2048, 1024, 16
D = C // H            # 64
NH = 8                # local heads per core
DG = 4                # d-groups of 128 partitions (2 heads each)
CK = 8                # contraction chunks of 128 over C
NQ = 4                # q tiles of 512
NT = 16               # t tiles of 128
QW = 512              # q tile width
KW = 128              # k tile width (partition dim of sT)
VW = D + 1            # v block incl ones column


# ---------------------------------------------------------------------------
# Tile compatibility patches for this walrus build: it accepts at most ONE
# sync wait per instruction, while TileContext attaches several.  Split the
# extras onto dedicated nops (same engine, just before the instruction).
# ---------------------------------------------------------------------------
def _install_patches():
    if getattr(tile.TileContext, "_wsplit_patched", False):
        return

    def _drain_and_barrier(self, tick_clock, wait_clock):
        drain_inst = self.nc.sync.drain()
        wait_clock.add_sem_waits(
            drain_inst.ins, ScopedClock({None: tick_clock.global_clock})
        )
        si = drain_inst.ins.sync_info
        waits = list(si.on_wait or []) if si is not None else []
        if len(waits) > 1:
            si.on_wait = waits[:1]
            for w in waits[1:]:
                n = self.nc.sync.nop(nofuse=True, hint="tail_wait")
                if n.ins.sync_info is None:
                    n.ins.sync_info = mybir.SyncInfo(on_wait=[w], on_update=[])
                else:
                    n.ins.sync_info.on_wait = [w]
        self.nc.all_engine_barrier()
        popped = self.nc._tile_sem_poison_stack.pop()
        assert popped is self._sem_poison
        self.nc.clear_and_free_semaphores(list(self.sems.allocated().values()))
        self.nc.all_engine_barrier()

    _orig_commit = tile.TileContext._commit_and_lower

    def _commit_and_lower(self, inst, original_block, old_bb_map, bb_to_exit_bb):
        si = getattr(inst, "sync_info", None)
        if si is not None and si.on_wait and len(si.on_wait) > 1:
            waits = list(si.on_wait)
            si.on_wait = [waits[-1]]
            for w in waits[:-1]:
                nop = self.nc.engines[inst.engine].nop(nofuse=True, hint="wsplit")
                if nop.ins.sync_info is None:
                    nop.ins.sync_info = mybir.SyncInfo(on_wait=[w], on_update=[])
                else:
                    nop.ins.sync_info.on_wait = [w]
        return _orig_commit(self, inst, original_block, old_bb_map, bb_to_exit_bb)

    tile.TileContext._drain_and_barrier = _drain_and_barrier
    tile.TileContext._commit_and_lower = _commit_and_lower
    tile.TileContext._wsplit_patched = True


# ---------------------------------------------------------------------------
# Kernel program
# ---------------------------------------------------------------------------
def _build_program():
    _install_patches()
    nc = bass.Bass()

    xT_e = nc.dram_tensor("xT", [C, T], F32, kind="ExternalInput")
    wq_e = nc.dram_tensor("wq", [C, NH * D], F32, kind="ExternalInput")
    wk_e = nc.dram_tensor("wk", [C, NH * D], F32, kind="ExternalInput")
    wv_e = nc.dram_tensor("wv", [C, NH * D], F32, kind="ExternalInput")
    bq_e = nc.dram_tensor("bq", [NH * D], F32, kind="ExternalInput")
    bk_e = nc.dram_tensor("bk", [NH * D], F32, kind="ExternalInput")
    bv_e = nc.dram_tensor("bv", [NH * D], F32, kind="ExternalInput")
    wp_e = nc.dram_tensor("wp", [NH * D, C], F32, kind="ExternalInput")
    mask_e = nc.dram_tensor("mask", [KW, 896], F32, kind="ExternalInput")
    sel_e = nc.dram_tensor("sel", [NH, NH * D], F32, kind="ExternalInput")
    eye_e = nc.dram_tensor("eye", [KW, KW + 1], F32, kind="ExternalInput")
    ones_e = nc.dram_tensor("ones1", [1, KW], F32, kind="ExternalInput")
    out_e = nc.dram_tensor("out", [T, C], F32, kind="ExternalOutput")

    with tile.TileContext(nc) as tc:
        with (
            tc.tile_pool(name="persist", bufs=1) as persist,
            tc.tile_pool(name="psum_y", bufs=2, space="PSUM") as psum_y,
        ):
            # causal mask as additive bias (0 visible / -480 masked),
            # applied on the PE by accumulating eye.T @ mask into sT psum.
            mask_sb = persist.tile([KW, 896], F32R, tag="mask")
            nc.sync.dma_start(mask_sb[:], mask_e[:].bitcast(F32R))
            # eye cols 0:128 = identity (causal-bias matmul lhsT);
            # col 128 = all-ones (v ones-column source)
            eye_sb = persist.tile([KW, KW + 1], F32R, tag="eye")
            nc.sync.dma_start(eye_sb[:], eye_e[:].bitcast(F32R))
            ones = persist.tile([1, KW], F32R, tag="ones")
            nc.sync.dma_start(ones[:], ones_e[:].bitcast(F32R))
            bv_sb = persist.tile([1, NH * D], F32R, tag="bv")
            nc.sync.dma_start(bv_sb[:], bv_e[None, :].bitcast(F32R))
            # selector kron(I8, ones64): bc matmul picks head h's recip row
            sel_sb = persist.tile([NH, NH * D], F32R, tag="sel")
            nc.sync.dma_start(sel_sb[:], sel_e[:].bitcast(F32R))
            # q/k biases in column layout [d_in_group, dg] for the fused
            # bias-add on the ScalarE psum->sbuf copy (per-partition bias).
            bqk_sb = persist.tile([128, 2 * DG], F32, tag="bqk")
            nc.sync.dma_start(
                bqk_sb[:, 0:DG], bq_e.rearrange("(dg p) -> p dg", p=128)
            )
            nc.sync.dma_start(
                bqk_sb[:, DG : 2 * DG], bk_e.rearrange("(dg p) -> p dg", p=128)
            )

            qT = persist.tile([128, DG, T], F32R, tag="qT")
            kT = persist.tile([128, DG, T], F32R, tag="kT")
            v_sb = persist.tile([128, NT, NH * VW], F32R, tag="v")

            # ---------------- Phase 1: QKV projections -------------------
            IDENT = mybir.ActivationFunctionType.Identity
            SW = 256  # x slab width (double-buffered)
            NS = T // SW
            with (
                tc.tile_pool(name="ph1", bufs=1) as ph1,
                tc.tile_pool(name="ph1x", bufs=2) as ph1x,
                tc.tile_pool(name="ph1ps", bufs=3, space="PSUM") as ph1ps,
                tc.tile_pool(name="ph1psv", bufs=2, space="PSUM") as ph1psv,
            ):
                wq_sb = ph1.tile([128, CK, NH * D], F32R, tag="wq")
                wk_sb = ph1.tile([128, CK, NH * D], F32R, tag="wk")
                wv_sb = ph1.tile([128, CK, NH * D], F32R, tag="wv")
                for w_sb, w_ext in ((wq_sb, wq_e), (wk_sb, wk_e), (wv_sb, wv_e)):
                    nc.sync.dma_start(
                        w_sb[:],
                        w_ext.rearrange("(ck p) d -> p ck d", p=128).bitcast(F32R),
                    )

                for th in range(NS):
                    xt = ph1x.tile([128, CK, SW], F32R, tag="xt")
                    nc.sync.dma_start(
                        xt[:],
                        xT_e[:, th * SW : (th + 1) * SW]
                        .rearrange("(ck p) t -> p ck t", p=128)
                        .bitcast(F32R),
                    )

                    # q and k (transposed layout [d, t]); bias fused into the
                    # ScalarE psum->sbuf copy (per-partition bias add).
                    for dst, w_sb, bc0 in ((qT, wq_sb, 0), (kT, wk_sb, DG)):
                        for dg in range(DG):
                            ps = ph1ps.tile([128, SW], F32, tag="ph1ps")
                            for ck in range(CK):
                                nc.tensor.matmul(
                                    ps[:],
                                    w_sb[:, ck, dg * 128 : (dg + 1) * 128],
                                    xt[:, ck, :],
                                    start=(ck == 0),
                                    stop=(ck == CK - 1),
                                )
                            nc.scalar.activation(
                                dst[:, dg, th * SW : th * SW + SW],
                                ps[:],
                                IDENT,
                                bias=bqk_sb[:, bc0 + dg : bc0 + dg + 1],
                            )

                    # v (natural layout [t, d]) + ones column
                    for tt in range(SW // 128):
                        tta = th * (SW // 128) + tt
                        ps = ph1psv.tile([128, NH * D], F32, tag="ph1psv")
                        for ck in range(CK):
                            nc.tensor.matmul(
                                ps[:],
                                xt[:, ck, tt * 128 : (tt + 1) * 128],
                                wv_sb[:, ck, :],
                                start=(ck == 0),
                                stop=False,
                            )
                        nc.tensor.matmul(
                            ps[:],
                            ones[0:1, 0:128],
                            bv_sb[:],
                            start=False,
                            stop=True,
                        )
                        v_blk = v_sb[:, tta, :].rearrange("p (h e) -> p h e", e=VW)
                        nc.vector.tensor_copy(
                            v_blk[:, :, 0:D],
                            ps[:].rearrange("p (h d) -> p h d", d=D).bitcast(F32R),
                        )
                        nc.vector.tensor_copy(
                            v_blk[:, :, D : D + 1],
                            eye_sb[:, KW : KW + 1].broadcast_to([KW, NH, 1]),
                        )

            # ------- Phase 2 + 3: attention, proj interleaved per qt ------
            yT = persist.tile([128, DG, T], F32R, tag="yT")
            with (
                tc.tile_pool(name="ph2", bufs=3) as ph2,
                tc.tile_pool(name="ph2ps", bufs=2, space="PSUM") as ph2ps,
                tc.tile_pool(name="ph2bc", bufs=2, space="PSUM") as ph2bc,
                tc.tile_pool(name="ph3", bufs=1) as ph3,
                tc.tile_pool(name="work", bufs=2) as work,
            ):
                wp_sb = ph3.tile([128, DG, C], F32R, tag="wp")
                nc.sync.dma_start(
                    wp_sb[:], wp_e.rearrange("(jc p) e -> p jc e", p=128).bitcast(F32R)
                )
                IDENT2 = mybir.ActivationFunctionType.Identity

                def norm_and_proj(qt, den_all):
                    # one batched reciprocal for all 8 heads of this qt
                    recip_all = ph2.tile([NH, QW], F32R, tag="recip", bufs=2)
                    with nc.allow_low_precision(reason="softmax reciprocal"):
                        nc.vector.reciprocal(recip_all[:], den_all[:])
                    for h in range(NH):
                        dg = h // 2
                        po = 64 * (h % 2)
                        bc_slot = ph2bc.tile([KW, QW], F32, tag="bcops")
                        bc_ps = bc_slot[0:D, 0:QW]
                        nc.tensor.matmul(
                            bc_ps,
                            sel_sb[:, h * D : (h + 1) * D],
                            recip_all[:],
                            start=True,
                            stop=True,
                        )
                        ysl = yT[po : po + 64, dg, qt * QW : (qt + 1) * QW]
                        nc.vector.tensor_mul(ysl, ysl, bc_ps.bitcast(F32R))
                    # output projection for this qt's t range
                    for tt in range(4 * qt, 4 * (qt + 1)):
                        for eh in range(2):
                            o_slot = ph2bc.tile([KW, QW], F32, tag="bcops")
                            o_ps = o_slot[:, 0:QW]
                            for jc in range(DG):
                                nc.tensor.matmul(
                                    o_ps,
                                    yT[:, jc, tt * 128 : (tt + 1) * 128],
                                    wp_sb[:, jc, eh * QW : (eh + 1) * QW],
                                    start=(jc == 0),
                                    stop=(jc == DG - 1),
                                )
                            o_sb = work.tile([128, QW], F32, tag="osb")
                            nc.vector.tensor_copy(o_sb[:], o_ps)
                            nc.sync.dma_start(
                                out_e[
                                    tt * 128 : (tt + 1) * 128,
                                    eh * QW : (eh + 1) * QW,
                                ],
                                o_sb[:],
                            )

                pending_qt = None
                for qt in range(NQ):
                    den_all = ph2.tile([NH, QW], F32, tag="den", bufs=2)
                    nkt = 4 * (qt + 1)
                    for hp in range(NH // 2):
                        # head pair: hA on partitions 0:64, hB on 64:128.
                        # Interleaved so adjacent sT matmuls use different PE
                        # row groups (concurrent) and exp(A) overlaps sT(B).
                        dg = hp
                        hA, hB = 2 * hp, 2 * hp + 1
                        qA = qT[0:64, dg, qt * QW : (qt + 1) * QW]
                        qB = qT[64:128, dg, qt * QW : (qt + 1) * QW]
                        y_psA = psum_y.tile([D + 1, QW], F32, tag="yps")
                        y_psB = psum_y.tile([D + 1, QW], F32, tag="yps")

                        def emit_y(pending_pr, ptA, ptB):
                            for hf in range(2):
                                kt = 2 * pending_pr + hf
                                for p_t, h, y_ps in (
                                    (ptA, hA, y_psA),
                                    (ptB, hB, y_psB),
                                ):
                                    nc.tensor.matmul(
                                        y_ps[:],
                                        v_sb[:, kt, h * VW : (h + 1) * VW],
                                        p_t[:, hf * QW : (hf + 1) * QW],
                                        start=(kt == 0),
                                        stop=(kt == nkt - 1),
                                    )

                        pending = None
                        for pr in range(nkt // 2):
                            spA = ph2ps.tile([KW, 2 * QW], F32, tag="sps")
                            spB = ph2ps.tile([KW, 2 * QW], F32, tag="sps")
                            for hf in range(2):
                                kt = 2 * pr + hf
                                ksl = slice(kt * KW, (kt + 1) * KW)
                                off = KW * kt - QW * qt
                                diag = off >= 0
                                for sp, lo in ((spA, 0), (spB, 64)):
                                    nc.tensor.matmul(
                                        sp[:, hf * QW : (hf + 1) * QW],
                                        kT[lo : lo + 64, dg, ksl],
                                        qA if lo == 0 else qB,
                                        start=True,
                                        stop=not diag,
                                    )
                                    if diag:  # accumulate -480 causal bias
                                        nc.tensor.matmul(
                                            sp[:, hf * QW : (hf + 1) * QW],
                                            eye_sb[:, 0:KW],
                                            mask_sb[:, 384 - off : 896 - off],
                                            start=False,
                                            stop=True,
                                        )
                            ptA = ph2.tile([KW, 2 * QW], F32R, tag="pt", bufs=4)
                            nc.scalar.activation(ptA[:], spA[:], EXP, scale=0.125)
                            ptB = ph2.tile([KW, 2 * QW], F32R, tag="pt", bufs=4)
                            nc.scalar.activation(ptB[:], spB[:], EXP, scale=0.125)
                            # software pipeline: y matmuls lag one iteration
                            # so PE never blocks on the current exp
                            if pending is not None:
                                emit_y(*pending)
                            pending = (pr, ptA, ptB)
                        emit_y(*pending)
                        # spill unnormalized y straight into yT (freeing the
                        # psum slots); normalization is applied in-place later
                        for h, y_ps, spill_eng in (
                            (hA, y_psA, "act"),
                            (hB, y_psB, "dve"),
                        ):
                            po = 64 * (h % 2)
                            ysl = yT[po : po + 64, dg, qt * QW : (qt + 1) * QW]
                            if spill_eng == "act":
                                nc.scalar.activation(ysl, y_ps[0:D, :], IDENT2)
                            else:
                                nc.vector.tensor_copy(ysl, y_ps[0:D, :].bitcast(F32R))
                            den_st = ph2.tile([1, QW], F32, tag="denst", bufs=1)
                            nc.vector.tensor_copy(den_st[:], y_ps[D : D + 1, :])
                            # spread den rows across partitions 0..7 (DMA can
                            # write arbitrary partitions; engines cannot)
                            nc.sync.dma_start(den_all[h : h + 1, :], den_st[:])

                    # qt-level software pipeline: normalize+proj of the
                    # previous qt runs while this qt's attention streams
                    if pending_qt is not None:
                        norm_and_proj(*pending_qt)
                    pending_qt = (qt, den_all)
                norm_and_proj(*pending_qt)
    return nc


_CACHE = {}
last_exec_time_ns = None


def _causal_mask_np():
    # additive bias: 0 where visible (m >= tk + 384), -480 where masked
    m = np.full((KW, 896), -480.0, dtype=np.float32)
    tk = np.arange(KW)[:, None]
    mm = np.arange(896)[None, :]
    m[mm >= tk + 384] = 0.0
    return m


def kernel(x, w_attn, b_attn, w_proj, b_proj):
    global last_exec_time_ns
    x = np.asarray(x, dtype=np.float32)
    w_attn = np.asarray(w_attn, dtype=np.float32)
    b_attn = np.asarray(b_attn, dtype=np.float32)
    w_proj = np.asarray(w_proj, dtype=np.float32)
    b_proj = np.asarray(b_proj, dtype=np.float32)

    if "nc" not in _CACHE:
        _CACHE["nc"] = _build_program()
    nc = _CACHE["nc"]

    mask = _causal_mask_np()
    in_maps = []
    for c in range(8):
        b, g = divmod(c, 2)
        s = slice(g * 512, (g + 1) * 512)
        in_maps.append(
            {
                "xT": np.ascontiguousarray(x[b].T),
                "wq": np.ascontiguousarray(w_attn[:, s]),
                "wk": np.ascontiguousarray(w_attn[:, 1024 + g * 512 : 1024 + (g + 1) * 512]),
                "wv": np.ascontiguousarray(w_attn[:, 2048 + g * 512 : 2048 + (g + 1) * 512]),
                "bq": np.ascontiguousarray(b_attn[s]),
                "bk": np.ascontiguousarray(b_attn[1024 + g * 512 : 1024 + (g + 1) * 512]),
                "bv": np.ascontiguousarray(b_attn[2048 + g * 512 : 2048 + (g + 1) * 512]),
                "wp": np.ascontiguousarray(w_proj[s, :]),
                "mask": mask,
                "sel": np.kron(np.eye(NH, dtype=np.float32), np.ones((1, D), np.float32)),
                "eye": np.concatenate(
                    [np.eye(KW, dtype=np.float32), np.ones((KW, 1), np.float32)],
                    axis=1,
                ),
                "ones1": np.ones((1, KW), dtype=np.float32),
            }
        )

    res = run_bass_kernel_spmd(nc, in_maps, core_ids=list(range(8)))
    last_exec_time_ns = res.exec_time_ns

    out = np.empty((B, T, C), dtype=np.float32)
    for b in range(B):
        out[b] = res.results[2 * b]["out"] + res.results[2 * b + 1]["out"] + b_proj
    return out



# revision 16
# speedup vs baseline: 1.1986x; 1.1986x over previous
"""Causal self-attention Trainium2 kernel (8 NeuronCores, SPMD).

Problem: B=4, T=2048, C=1024, H=16 heads, D=64.
  qkv = x @ w_attn + b_attn ; causal softmax attention ; out = y @ w_proj + b_proj

Sharding: core c = 2*b + g  handles batch b with head-group g (heads 8g..8g+7).
Each core computes a partial projection output (its 8 heads' contribution);
the host sums the two partials per batch and adds b_proj + b_v @ w_proj
(the v-bias commutes through softmax normalization: sum_k p*(v+bv)/den =
y/den + bv, so it folds into a constant output bias added host-side).

All matmul operands are bf16 (psum accumulation stays fp32): halves the
PE weight-load (LDWEIGHTS) time so stationary reloads hide under the
previous matmul, halves HBM traffic, and keeps full 1 row/cycle rate
even for <256-wide outputs.

Per-core layouts:
  xT  [1024, 2048] bf16 = x[b].T            (contraction dim on partitions)
  w_q/w_k/w_v [1024, 512] bf16, b_q/b_k [512] f32 (head-group slices)
  w_p [512, 1024] bf16                      (head-group rows of w_proj)
  mask [128, 896] bf16  mask[tk, m] = 0 if m >= tk+384 else -480
  eye  [128, 128] bf16  identity (lhsT for mask-accumulate matmuls)

On-chip:
  qT, kT  [128, 4, 2048] bf16 (d-group, t) - head pair per 128 partitions
  v       [128, 16, 8*65] bf16 (t-tile, 8 heads x (64 v | 1 ones))
  sT tile [128 tk, 512 tq] psum = kT.T @ qT ; causal handling per k-tile
  offset off = 128*kt - 512*qt:
    off < 0   : plain matmul
    off 0/128 : matmul + eye@mask accumulate on first off+128 cols only
    off >= 256: eye@mask writes psum first (start=True), then s-matmul
                adds only cols [off:512]  (cols [0:off] fully masked)
  p = exp(s/8) via ScalarE -> bf16
  y_psum [65, 512] accumulates [v|1].T @ p over kt (diag k-tiles only
  touch cols [off:512]); row 64 = softmax denominator
  den rows gathered to den_all[8,512] (Pool-queue DMAs), one batched
  reciprocal per qt, gpsimd partition_broadcast spreads recip rows, DVE
  multiplies yT in place; out = yT.T @ w_p, stored bf16.
"""

import sys
import os

sys.path.insert(0, "/opt/trn_rl_repo")

import numpy as np
import concourse.bass as bass
import concourse.mybir as mybir
import concourse.tile as tile
from concourse import library_config
from concourse.vector_clock import ScopedClock
from concourse.bass_utils import run_bass_kernel_spmd

F32 = mybir.dt.float32
BF16 = mybir.dt.bfloat16
EXP = mybir.ActivationFunctionType.Exp

B, T, C, H = 4, 2048, 1024, 16
D = C // H            # 64
NH = 8                # local heads per core
DG = 4                # d-groups of 128 partitions (2 heads each)
CK = 8                # contraction chunks of 128 over C
NQ = 4                # q tiles of 512
NT = 16               # t tiles of 128
QW = 512              # q tile width
KW = 128              # k tile width (partition dim of sT)
VW = D + 1            # v block incl ones column

BF16NP = mybir.dt.np(BF16)


# ---------------------------------------------------------------------------
# Tile compatibility patches for this walrus build: it accepts at most ONE
# sync wait per instruction, while TileContext attaches several.  Split the
# extras onto dedicated nops (same engine, just before the instruction).
# ---------------------------------------------------------------------------
def _install_patches():
    if getattr(tile.TileContext, "_wsplit_patched", False):
        return

    def _drain_and_barrier(self, tick_clock, wait_clock):
        drain_inst = self.nc.sync.drain()
        wait_clock.add_sem_waits(
            drain_inst.ins, ScopedClock({None: tick_clock.global_clock})
        )
        si = drain_inst.ins.sync_info
        waits = list(si.on_wait or []) if si is not None else []
        if len(waits) > 1:
            si.on_wait = waits[:1]
            for w in waits[1:]:
                n = self.nc.sync.nop(nofuse=True, hint="tail_wait")
                if n.ins.sync_info is None:
                    n.ins.sync_info = mybir.SyncInfo(on_wait=[w], on_update=[])
                else:
                    n.ins.sync_info.on_wait = [w]
        self.nc.all_engine_barrier()
        popped = self.nc._tile_sem_poison_stack.pop()
        assert popped is self._sem_poison
        self.nc.clear_and_free_semaphores(list(self.sems.allocated().values()))
        self.nc.all_engine_barrier()

    _orig_commit = tile.TileContext._commit_and_lower

    def _commit_and_lower(self, inst, original_block, old_bb_map, bb_to_exit_bb):
        si = getattr(inst, "sync_info", None)
        if si is not None and si.on_wait and len(si.on_wait) > 1:
            waits = list(si.on_wait)
            si.on_wait = [waits[-1]]
            for w in waits[:-1]:
                nop = self.nc.engines[inst.engine].nop(nofuse=True, hint="wsplit")
                if nop.ins.sync_info is None:
                    nop.ins.sync_info = mybir.SyncInfo(on_wait=[w], on_update=[])
                else:
                    nop.ins.sync_info.on_wait = [w]
        return _orig_commit(self, inst, original_block, old_bb_map, bb_to_exit_bb)

    tile.TileContext._drain_and_barrier = _drain_and_barrier
    tile.TileContext._commit_and_lower = _commit_and_lower
    tile.TileContext._wsplit_patched = True


# ---------------------------------------------------------------------------
# Kernel program
# ---------------------------------------------------------------------------
def _build_program():
    _install_patches()
    nc = bass.Bass()

    xT_e = nc.dram_tensor("xT", [C, T], BF16, kind="ExternalInput")
    wq_e = nc.dram_tensor("wq", [C, NH * D], BF16, kind="ExternalInput")
    wk_e = nc.dram_tensor("wk", [C, NH * D], BF16, kind="ExternalInput")
    wv_e = nc.dram_tensor("wv", [C, NH * D], BF16, kind="ExternalInput")
    bq_e = nc.dram_tensor("bq", [NH * D], F32, kind="ExternalInput")
    bk_e = nc.dram_tensor("bk", [NH * D], F32, kind="ExternalInput")
    wp_e = nc.dram_tensor("wp", [NH * D, C], BF16, kind="ExternalInput")
    mask_e = nc.dram_tensor("mask", [KW, 896], BF16, kind="ExternalInput")
    eye_e = nc.dram_tensor("eye", [KW, KW], BF16, kind="ExternalInput")
    sel_e = nc.dram_tensor("sel", [NH, NH * D], BF16, kind="ExternalInput")
    out_e = nc.dram_tensor("out", [T, C], BF16, kind="ExternalOutput")

    with tile.TileContext(nc) as tc:
        with (
            tc.tile_pool(name="persist", bufs=1) as persist,
            tc.tile_pool(name="psum_y", bufs=2, space="PSUM") as psum_y,
        ):
            # causal mask as additive bias (0 visible / -480 masked),
            # applied on the PE by accumulating eye.T @ mask into sT psum.
            mask_sb = persist.tile([KW, 896], BF16, tag="mask")
            nc.sync.dma_start(mask_sb[:], mask_e[:])
            eye_sb = persist.tile([KW, KW], BF16, tag="eye")
            nc.sync.dma_start(eye_sb[:], eye_e[:])
            # selector kron(I8, ones64): bc matmul picks head h's recip row
            sel_sb = persist.tile([NH, NH * D], BF16, tag="sel")
            nc.sync.dma_start(sel_sb[:], sel_e[:])
            # q/k biases in column layout [d_in_group, dg] for the fused
            # bias-add on the DVE psum->sbuf spill (per-partition bias).
            bqk_sb = persist.tile([128, 2 * DG], F32, tag="bqk")
            nc.sync.dma_start(
                bqk_sb[:, 0:DG], bq_e.rearrange("(dg p) -> p dg", p=128)
            )
            nc.sync.dma_start(
                bqk_sb[:, DG : 2 * DG], bk_e.rearrange("(dg p) -> p dg", p=128)
            )

            qT = persist.tile([128, DG, T], BF16, tag="qT")
            kT = persist.tile([128, DG, T], BF16, tag="kT")
            v_sb = persist.tile([128, NT, NH * VW], BF16, tag="v")
            # ones column (softmax denominator source), written once
            nc.vector.memset(
                v_sb[:].rearrange("p t (h e) -> p t h e", e=VW)[:, :, :, D : D + 1],
                1.0,
            )

            # ---------------- Phase 1: QKV projections -------------------
            SW = 256  # x slab width (double-buffered)
            NS = T // SW
            with (
                tc.tile_pool(name="ph1", bufs=1) as ph1,
                tc.tile_pool(name="ph1x", bufs=2) as ph1x,
                tc.tile_pool(name="ph1ps", bufs=3, space="PSUM") as ph1ps,
                tc.tile_pool(name="ph1psv", bufs=2, space="PSUM") as ph1psv,
            ):
                wq_sb = ph1.tile([128, CK, NH * D], BF16, tag="wq")
                wk_sb = ph1.tile([128, CK, NH * D], BF16, tag="wk")
                wv_sb = ph1.tile([128, CK, NH * D], BF16, tag="wv")
                for w_sb, w_ext in ((wq_sb, wq_e), (wk_sb, wk_e), (wv_sb, wv_e)):
                    nc.sync.dma_start(
                        w_sb[:],
                        w_ext.rearrange("(ck p) d -> p ck d", p=128),
                    )

                for th in range(NS):
                    xt = ph1x.tile([128, CK, SW], BF16, tag="xt")
                    nc.sync.dma_start(
                        xt[:],
                        xT_e[:, th * SW : (th + 1) * SW].rearrange(
                            "(ck p) t -> p ck t", p=128
                        ),
                    )

                    # q and k (transposed layout [d, t]); bias fused into the
                    # DVE psum->sbuf spill (per-partition bias add + bf16 cast)
                    for dst, w_sb, bc0 in ((qT, wq_sb, 0), (kT, wk_sb, DG)):
                        for dg in range(DG):
                            ps = ph1ps.tile([128, SW], F32, tag="ph1ps")
                            for ck in range(CK):
                                nc.tensor.matmul(
                                    ps[:],
                                    w_sb[:, ck, dg * 128 : (dg + 1) * 128],
                                    xt[:, ck, :],
                                    start=(ck == 0),
                                    stop=(ck == CK - 1),
                                )
                            nc.vector.tensor_scalar_add(
                                dst[:, dg, th * SW : th * SW + SW],
                                ps[:],
                                bqk_sb[:, bc0 + dg : bc0 + dg + 1],
                            )

                    # v (natural layout [t, d]); no bias (folded host-side)
                    for tt in range(SW // 128):
                        tta = th * (SW // 128) + tt
                        ps = ph1psv.tile([128, NH * D], F32, tag="ph1psv")
                        for ck in range(CK):
                            nc.tensor.matmul(
                                ps[:],
                                xt[:, ck, tt * 128 : (tt + 1) * 128],
                                wv_sb[:, ck, :],
                                start=(ck == 0),
                                stop=(ck == CK - 1),
                            )
                        v_blk = v_sb[:, tta, :].rearrange("p (h e) -> p h e", e=VW)
                        nc.vector.tensor_copy(
                            v_blk[:, :, 0:D],
                            ps[:].rearrange("p (h d) -> p h d", d=D),
                        )

            # ------- Phase 2 + 3: attention, proj interleaved per qt ------
            yT = persist.tile([128, DG, T], BF16, tag="yT")
            with (
                tc.tile_pool(name="ph2", bufs=3) as ph2,
                tc.tile_pool(name="ph2ps", bufs=2, space="PSUM") as ph2ps,
                tc.tile_pool(name="projps", bufs=2, space="PSUM") as projps,
                tc.tile_pool(name="ph3", bufs=1) as ph3,
                tc.tile_pool(name="work", bufs=2) as work,
            ):
                wp_sb = ph3.tile([128, DG, C], BF16, tag="wp")
                nc.sync.dma_start(
                    wp_sb[:], wp_e.rearrange("(jc p) e -> p jc e", p=128)
                )

                def norm_and_proj(qt, den_all):
                    # one batched reciprocal for all 8 heads of this qt
                    recip_all = ph2.tile([NH, QW], BF16, tag="recip", bufs=2)
                    with nc.allow_low_precision(reason="softmax reciprocal"):
                        nc.vector.reciprocal(recip_all[:], den_all[:])
                    for h in range(NH):
                        dg = h // 2
                        po = 64 * (h % 2)
                        # broadcast recip row h across 64 partitions on the PE
                        # (sel matmul); engines can't address base partition h
                        bc_slot = projps.tile([KW, QW], F32, tag="ops")
                        bc_ps = bc_slot[0:D, 0:QW]
                        nc.tensor.matmul(
                            bc_ps,
                            sel_sb[:, h * D : (h + 1) * D],
                            recip_all[:],
                            start=True,
                            stop=True,
                        )
                        ysl = yT[po : po + 64, dg, qt * QW : (qt + 1) * QW]
                        nc.vector.tensor_mul(ysl, ysl, bc_ps)
                    # output projection for this qt's t range
                    for tt in range(4 * qt, 4 * (qt + 1)):
                        for eh in range(2):
                            o_ps = projps.tile([KW, QW], F32, tag="ops")
                            for jc in range(DG):
                                nc.tensor.matmul(
                                    o_ps[:],
                                    yT[:, jc, tt * 128 : (tt + 1) * 128],
                                    wp_sb[:, jc, eh * QW : (eh + 1) * QW],
                                    start=(jc == 0),
                                    stop=(jc == DG - 1),
                                )
                            o_sb = work.tile([128, QW], BF16, tag="osb")
                            nc.vector.tensor_copy(o_sb[:], o_ps[:])
                            nc.sync.dma_start(
                                out_e[
                                    tt * 128 : (tt + 1) * 128,
                                    eh * QW : (eh + 1) * QW,
                                ],
                                o_sb[:],
                            )

                pending_qt = None
                for qt in range(NQ):
                    den_all = ph2.tile([NH, QW], F32, tag="den", bufs=2)
                    nkt = 4 * (qt + 1)
                    for hp in range(NH // 2):
                        # head pair: hA on partitions 0:64, hB on 64:128.
                        dg = hp
                        hA, hB = 2 * hp, 2 * hp + 1
                        qA = qT[0:64, dg, qt * QW : (qt + 1) * QW]
                        qB = qT[64:128, dg, qt * QW : (qt + 1) * QW]
                        y_psA = psum_y.tile([D + 1, QW], F32, tag="yps")
                        y_psB = psum_y.tile([D + 1, QW], F32, tag="yps")

                        def emit_y(pending_pr, ptA, ptB):
                            for hf in range(2):
                                kt = 2 * pending_pr + hf
                                off = max(KW * kt - QW * qt, 0)
                                for p_t, h, y_ps in (
                                    (ptA, hA, y_psA),
                                    (ptB, hB, y_psB),
                                ):
                                    # diag k-tiles: p cols [0:off] are zero
                                    nc.tensor.matmul(
                                        y_ps[:, off:QW],
                                        v_sb[:, kt, h * VW : (h + 1) * VW],
                                        p_t[:, hf * QW + off : (hf + 1) * QW],
                                        start=(kt == 0),
                                        stop=(kt == nkt - 1),
                                        skip_group_check=True,
                                    )

                        pending = None
                        for pr in range(nkt // 2):
                            spA = ph2ps.tile([KW, 2 * QW], F32, tag="sps")
                            spB = ph2ps.tile([KW, 2 * QW], F32, tag="sps")
                            for hf in range(2):
                                kt = 2 * pr + hf
                                ksl = slice(kt * KW, (kt + 1) * KW)
                                off = KW * kt - QW * qt
                                for sp, lo, qfull in ((spA, 0, qA), (spB, 64, qB)):
                                    scol = slice(hf * QW, (hf + 1) * QW)
                                    if off < 256:
                                        # s first; small mask-add after
                                        nc.tensor.matmul(
                                            sp[:, scol],
                                            kT[lo : lo + 64, dg, ksl],
                                            qfull,
                                            start=True,
                                            stop=(off < 0),
                                        )
                                        if off >= 0:
                                            mw = off + KW  # masked col extent
                                            nc.tensor.matmul(
                                                sp[:, hf * QW : hf * QW + mw],
                                                eye_sb[:],
                                                mask_sb[:, 384 - off : 384 - off + mw],
                                                start=False,
                                                stop=True,
                                                skip_group_check=True,
                                            )
                                    else:
                                        # cols [0:off] fully masked: write the
                                        # full mask first, add s on the rest
                                        nc.tensor.matmul(
                                            sp[:, scol],
                                            eye_sb[:],
                                            mask_sb[:, 384 - off : 896 - off],
                                            start=True,
                                            stop=False,
                                        )
                                        nc.tensor.matmul(
                                            sp[:, hf * QW + off : (hf + 1) * QW],
                                            kT[lo : lo + 64, dg, ksl],
                                            qT[
                                                lo : lo + 64,
                                                dg,
                                                qt * QW + off : (qt + 1) * QW,
                                            ],
                                            start=False,
                                            stop=True,
                                            skip_group_check=True,
                                        )
                            ptA = ph2.tile([KW, 2 * QW], BF16, tag="pt", bufs=4)
                            nc.scalar.activation(ptA[:], spA[:], EXP, scale=0.125)
                            ptB = ph2.tile([KW, 2 * QW], BF16, tag="pt", bufs=4)
                            nc.scalar.activation(ptB[:], spB[:], EXP, scale=0.125)
                            # software pipeline: y matmuls lag one iteration
                            # so PE never blocks on the current exp
                            if pending is not None:
                                emit_y(*pending)
                            pending = (pr, ptA, ptB)
                        emit_y(*pending)
                        # spill unnormalized y straight into yT (freeing the
                        # psum slots); normalization is applied in-place later
                        for h, y_ps in ((hA, y_psA), (hB, y_psB)):
                            po = 64 * (h % 2)
                            ysl = yT[po : po + 64, dg, qt * QW : (qt + 1) * QW]
                            nc.vector.tensor_copy(ysl, y_ps[0:D, :])
                            den_st = ph2.tile([1, QW], F32, tag="denst", bufs=2)
                            nc.vector.tensor_copy(den_st[:], y_ps[D : D + 1, :])
                            # spread den rows across partitions 0..7 (DMA can
                            # write arbitrary partitions; engines cannot).
                            # Pool queue: cheap DMA issue (~36ns vs 565ns on SP)
                            nc.gpsimd.dma_start(den_all[h : h + 1, :], den_st[:])

                    # qt-level software pipeline: normalize+proj of the
                    # previous qt runs while this qt's attention streams
                    if pending_qt is not None:
                        norm_and_proj(*pending_qt)
                    pending_qt = (qt, den_all)
                norm_and_proj(*pending_qt)
    return nc


_CACHE = {}
last_exec_time_ns = None


def _causal_mask_np():
    # additive bias: 0 where visible (m >= tk + 384), -480 where masked
    m = np.full((KW, 896), -480.0, dtype=np.float32)
    tk = np.arange(KW)[:, None]
    mm = np.arange(896)[None, :]
    m[mm >= tk + 384] = 0.0
    return m.astype(BF16NP)


def kernel(x, w_attn, b_attn, w_proj, b_proj):
    global last_exec_time_ns
    x = np.asarray(x, dtype=np.float32)
    w_attn = np.asarray(w_attn, dtype=np.float32)
    b_attn = np.asarray(b_attn, dtype=np.float32)
    w_proj = np.asarray(w_proj, dtype=np.float32)
    b_proj = np.asarray(b_proj, dtype=np.float32)

    if "nc" not in _CACHE:
        _CACHE["nc"] = _build_program()
    nc = _CACHE["nc"]

    mask = _causal_mask_np()
    eye = np.eye(KW, dtype=np.float32).astype(BF16NP)
    sel = np.kron(
        np.eye(NH, dtype=np.float32), np.ones((1, D), np.float32)
    ).astype(BF16NP)
    in_maps = []
    for c in range(8):
        b, g = divmod(c, 2)
        s = slice(g * 512, (g + 1) * 512)
        in_maps.append(
            {
                "xT": np.ascontiguousarray(x[b].T).astype(BF16NP),
                "wq": np.ascontiguousarray(w_attn[:, s]).astype(BF16NP),
                "wk": np.ascontiguousarray(
                    w_attn[:, 1024 + g * 512 : 1024 + (g + 1) * 512]
                ).astype(BF16NP),
                "wv": np.ascontiguousarray(
                    w_attn[:, 2048 + g * 512 : 2048 + (g + 1) * 512]
                ).astype(BF16NP),
                "bq": np.ascontiguousarray(b_attn[s]),
                "bk": np.ascontiguousarray(b_attn[1024 + g * 512 : 1024 + (g + 1) * 512]),
                "wp": np.ascontiguousarray(w_proj[s, :]).astype(BF16NP),
                "mask": mask,
                "eye": eye,
                "sel": sel,
            }
        )

    res = run_bass_kernel_spmd(nc, in_maps, core_ids=list(range(8)))
    last_exec_time_ns = res.exec_time_ns

    # v-bias folds through softmax into a constant output bias (exact):
    # out += b_v @ w_proj, summed over both head-groups = full b_v @ w_proj
    bias_full = b_proj + b_attn[2048:].astype(np.float64) @ w_proj.astype(np.float64)
    bias_full = bias_full.astype(np.float32)

    out = np.empty((B, T, C), dtype=np.float32)
    for b in range(B):
        out[b] = (
            res.results[2 * b]["out"].astype(np.float32)
            + res.results[2 * b + 1]["out"].astype(np.float32)
            + bias_full
        )
    return out


# revision 18
# speedup vs baseline: 1.2673x; 1.0573x over previous
"""Causal self-attention Trainium2 kernel (8 NeuronCores, SPMD).

Problem: B=4, T=2048, C=1024, H=16 heads, D=64.
  qkv = x @ w_attn + b_attn ; causal softmax attention ; out = y @ w_proj + b_proj

Sharding: core c = 2*b + g  handles batch b with head-group g (heads 8g..8g+7).
Each core computes a partial projection output (its 8 heads' contribution);
the host sums the two partials per batch and adds b_proj + b_v @ w_proj
(the v-bias commutes through softmax normalization, so it folds into a
constant output bias added host-side).

All matmul operands are bf16 (psum accumulation stays fp32).

The kernel is software-pipelined at two levels to keep the PE dense (the
TRN2 PE p-state throttles after idle gaps, so PE must never starve):
  - within an attention row: y matmuls lag the exp by one iteration
  - across phases: QKV slab projections and the previous row's
    normalize+project work are interleaved between the attention
    iterations, filling the PE while ScalarE computes exp.

Row qt (q tokens [512qt, 512qt+512)) needs k/v tokens < 512(qt+1) =
slabs 0..2qt+1, so row qt runs while slabs 2qt+2, 2qt+3 are projected.

Causal handling per k-tile, offset off = 128*kt - 512*qt:
  off < 0   : plain matmul
  off 0/128 : matmul + eye@mask accumulate on first off+128 cols only
  off >= 256: eye@mask writes psum first (start=True), then the s-matmul
              adds only cols [off:512]  (cols [0:off] fully masked)
y matmuls on diag k-tiles touch only cols [off:512] (p is 0 below).
"""

import sys
import os

sys.path.insert(0, "/opt/trn_rl_repo")

import numpy as np
import concourse.bass as bass
import concourse.mybir as mybir
import concourse.tile as tile
from concourse.vector_clock import ScopedClock
from concourse.bass_utils import run_bass_kernel_spmd

F32 = mybir.dt.float32
BF16 = mybir.dt.bfloat16
EXP = mybir.ActivationFunctionType.Exp

B, T, C, H = 4, 2048, 1024, 16
D = C // H            # 64
NH = 8                # local heads per core
DG = 4                # d-groups of 128 partitions (2 heads each)
CK = 8                # contraction chunks of 128 over C
NQ = 4                # q tiles of 512
NT = 16               # t tiles of 128
QW = 512              # q tile width
KW = 128              # k tile width (partition dim of sT)
VW = D + 1            # v block incl ones column
SW = 256              # x slab width
NS = T // SW          # 8 slabs

BF16NP = mybir.dt.np(BF16)


# ---------------------------------------------------------------------------
# Tile compatibility patches for this walrus build: it accepts at most ONE
# sync wait per instruction, while TileContext attaches several.  Split the
# extras onto dedicated nops (same engine, just before the instruction).
# ---------------------------------------------------------------------------
def _install_patches():
    if getattr(tile.TileContext, "_wsplit_patched", False):
        return

    def _drain_and_barrier(self, tick_clock, wait_clock):
        drain_inst = self.nc.sync.drain()
        wait_clock.add_sem_waits(
            drain_inst.ins, ScopedClock({None: tick_clock.global_clock})
        )
        si = drain_inst.ins.sync_info
        waits = list(si.on_wait or []) if si is not None else []
        if len(waits) > 1:
            si.on_wait = waits[:1]
            for w in waits[1:]:
                n = self.nc.sync.nop(nofuse=True, hint="tail_wait")
                if n.ins.sync_info is None:
                    n.ins.sync_info = mybir.SyncInfo(on_wait=[w], on_update=[])
                else:
                    n.ins.sync_info.on_wait = [w]
        self.nc.all_engine_barrier()
        popped = self.nc._tile_sem_poison_stack.pop()
        assert popped is self._sem_poison
        self.nc.clear_and_free_semaphores(list(self.sems.allocated().values()))
        self.nc.all_engine_barrier()

    _orig_commit = tile.TileContext._commit_and_lower

    def _commit_and_lower(self, inst, original_block, old_bb_map, bb_to_exit_bb):
        si = getattr(inst, "sync_info", None)
        if si is not None and si.on_wait and len(si.on_wait) > 1:
            waits = list(si.on_wait)
            si.on_wait = [waits[-1]]
            for w in waits[:-1]:
                nop = self.nc.engines[inst.engine].nop(nofuse=True, hint="wsplit")
                if nop.ins.sync_info is None:
                    nop.ins.sync_info = mybir.SyncInfo(on_wait=[w], on_update=[])
                else:
                    nop.ins.sync_info.on_wait = [w]
        return _orig_commit(self, inst, original_block, old_bb_map, bb_to_exit_bb)

    tile.TileContext._drain_and_barrier = _drain_and_barrier
    tile.TileContext._commit_and_lower = _commit_and_lower
    tile.TileContext._wsplit_patched = True


# ---------------------------------------------------------------------------
# Kernel program
# ---------------------------------------------------------------------------
def _build_program():
    _install_patches()
    nc = bass.Bass()

    xT_e = nc.dram_tensor("xT", [C, T], BF16, kind="ExternalInput")
    wq_e = nc.dram_tensor("wq", [C, NH * D], BF16, kind="ExternalInput")
    wk_e = nc.dram_tensor("wk", [C, NH * D], BF16, kind="ExternalInput")
    wv_e = nc.dram_tensor("wv", [C, NH * D], BF16, kind="ExternalInput")
    bq_e = nc.dram_tensor("bq", [NH * D], F32, kind="ExternalInput")
    bk_e = nc.dram_tensor("bk", [NH * D], F32, kind="ExternalInput")
    wp_e = nc.dram_tensor("wp", [NH * D, C], BF16, kind="ExternalInput")
    mask_e = nc.dram_tensor("mask", [KW, 896], BF16, kind="ExternalInput")
    eye_e = nc.dram_tensor("eye", [KW, KW], BF16, kind="ExternalInput")
    sel_e = nc.dram_tensor("sel", [NH, NH * D], BF16, kind="ExternalInput")
    out_e = nc.dram_tensor("out", [T, C], BF16, kind="ExternalOutput")

    with tile.TileContext(nc) as tc:
        with (
            tc.tile_pool(name="persist", bufs=1) as persist,
            tc.tile_pool(name="ph1x", bufs=2) as ph1x,
            tc.tile_pool(name="ph2", bufs=3) as ph2,
            tc.tile_pool(name="work", bufs=2) as work,
            tc.tile_pool(name="ph2ps", bufs=2, space="PSUM") as ph2ps,
            tc.tile_pool(name="psum_y", bufs=2, space="PSUM") as psum_y,
            tc.tile_pool(name="opsps", bufs=2, space="PSUM") as opsps,
        ):
            mask_sb = persist.tile([KW, 896], BF16, tag="mask")
            nc.sync.dma_start(mask_sb[:], mask_e[:])
            eye_sb = persist.tile([KW, KW], BF16, tag="eye")
            nc.sync.dma_start(eye_sb[:], eye_e[:])
            # selector kron(I8, ones64): bc matmul picks head h's recip row
            sel_sb = persist.tile([NH, NH * D], BF16, tag="sel")
            nc.sync.dma_start(sel_sb[:], sel_e[:])
            # q/k biases in column layout [d_in_group, dg] for the fused
            # bias-add on the DVE psum->sbuf spill (per-partition bias).
            bqk_sb = persist.tile([128, 2 * DG], F32, tag="bqk")
            nc.sync.dma_start(
                bqk_sb[:, 0:DG], bq_e.rearrange("(dg p) -> p dg", p=128)
            )
            nc.sync.dma_start(
                bqk_sb[:, DG : 2 * DG], bk_e.rearrange("(dg p) -> p dg", p=128)
            )

            wq_sb = persist.tile([128, CK, NH * D], BF16, tag="wq")
            wk_sb = persist.tile([128, CK, NH * D], BF16, tag="wk")
            wv_sb = persist.tile([128, CK, NH * D], BF16, tag="wv")
            for w_sb, w_ext in ((wq_sb, wq_e), (wk_sb, wk_e), (wv_sb, wv_e)):
                nc.sync.dma_start(
                    w_sb[:], w_ext.rearrange("(ck p) d -> p ck d", p=128)
                )
            wp_sb = persist.tile([128, DG, C], BF16, tag="wp")
            nc.sync.dma_start(wp_sb[:], wp_e.rearrange("(jc p) e -> p jc e", p=128))

            qT = persist.tile([128, DG, T], BF16, tag="qT")
            kT = persist.tile([128, DG, T], BF16, tag="kT")
            v_sb = persist.tile([128, NT, NH * VW], BF16, tag="v")
            yT = persist.tile([128, DG, T], BF16, tag="yT")
            # ones column (softmax denominator source), written once
            nc.vector.memset(
                v_sb[:].rearrange("p t (h e) -> p t h e", e=VW)[:, :, :, D : D + 1],
                1.0,
            )

            # ---------- work items: QKV slab pieces -----------------------
            xt_tiles = {}

            def slab_items(th):
                def dma_item():
                    xt = ph1x.tile([128, CK, SW], BF16, tag="xt")
                    xt_tiles[th] = xt
                    nc.sync.dma_start(
                        xt[:],
                        xT_e[:, th * SW : (th + 1) * SW].rearrange(
                            "(ck p) t -> p ck t", p=128
                        ),
                    )

                items = [dma_item]

                def qk_item(dst, w_sb, bc0, dg):
                    def run():
                        xt = xt_tiles[th]
                        ps = opsps.tile([KW, QW], F32, tag="ops")
                        for ck in range(CK):
                            nc.tensor.matmul(
                                ps[:, 0:SW],
                                w_sb[:, ck, dg * 128 : (dg + 1) * 128],
                                xt[:, ck, :],
                                start=(ck == 0),
                                stop=(ck == CK - 1),
                            )
                        nc.vector.tensor_scalar_add(
                            dst[:, dg, th * SW : th * SW + SW],
                            ps[:, 0:SW],
                            bqk_sb[:, bc0 + dg : bc0 + dg + 1],
                        )

                    return run

                for dst, w_sb, bc0 in ((qT, wq_sb, 0), (kT, wk_sb, DG)):
                    for dg in range(DG):
                        items.append(qk_item(dst, w_sb, bc0, dg))

                def v_item(tt):
                    def run():
                        xt = xt_tiles[th]
                        tta = th * (SW // 128) + tt
                        ps = opsps.tile([KW, QW], F32, tag="ops")
                        for ck in range(CK):
                            nc.tensor.matmul(
                                ps[:],
                                xt[:, ck, tt * 128 : (tt + 1) * 128],
                                wv_sb[:, ck, :],
                                start=(ck == 0),
                                stop=(ck == CK - 1),
                            )
                        v_blk = v_sb[:, tta, :].rearrange("p (h e) -> p h e", e=VW)
                        nc.vector.tensor_copy(
                            v_blk[:, :, 0:D],
                            ps[:].rearrange("p (h d) -> p h d", d=D),
                        )

                    return run

                for tt in range(SW // 128):
                    items.append(v_item(tt))
                return items

            # ---------- work items: normalize + project row nq ------------
            def norm_proj_items(nq, den_all):
                recip_all = ph2.tile([NH, QW], BF16, tag="recip", bufs=2)

                def recip_item():
                    with nc.allow_low_precision(reason="softmax reciprocal"):
                        nc.vector.reciprocal(recip_all[:], den_all[:])

                items = [recip_item]

                def norm_item(h):
                    def run():
                        dg = h // 2
                        po = 64 * (h % 2)
                        # broadcast recip row h across 64 partitions on the
                        # PE; engines can't address base partition h
                        bc_slot = opsps.tile([KW, QW], F32, tag="ops")
                        bc_ps = bc_slot[0:D, 0:QW]
                        nc.tensor.matmul(
                            bc_ps,
                            sel_sb[:, h * D : (h + 1) * D],
                            recip_all[:],
                            start=True,
                            stop=True,
                        )
                        ysl = yT[po : po + 64, dg, nq * QW : (nq + 1) * QW]
                        nc.vector.tensor_mul(ysl, ysl, bc_ps)

                    return run

                for h in range(NH):
                    items.append(norm_item(h))

                def proj_item(tt, eh):
                    def run():
                        o_ps = opsps.tile([KW, QW], F32, tag="ops")
                        for jc in range(DG):
                            nc.tensor.matmul(
                                o_ps[:],
                                yT[:, jc, tt * 128 : (tt + 1) * 128],
                                wp_sb[:, jc, eh * QW : (eh + 1) * QW],
                                start=(jc == 0),
                                stop=(jc == DG - 1),
                            )
                        o_sb = work.tile([128, QW], BF16, tag="osb")
                        nc.vector.tensor_copy(o_sb[:], o_ps[:])
                        nc.sync.dma_start(
                            out_e[
                                tt * 128 : (tt + 1) * 128, eh * QW : (eh + 1) * QW
                            ],
                            o_sb[:],
                        )

                    return run

                for tt in range(4 * nq, 4 * (nq + 1)):
                    for eh in range(2):
                        items.append(proj_item(tt, eh))
                return items

            # ---------- prologue: slabs 0,1 -------------------------------
            for it in slab_items(0) + slab_items(1):
                it()

            # ---------- attention rows with interleaved filler ------------
            pending_norm = None  # (nq, den_all) of previous row
            for qt in range(NQ):
                filler = []
                if qt < NQ - 1:
                    filler += slab_items(2 * qt + 2) + slab_items(2 * qt + 3)
                if pending_norm is not None:
                    filler += norm_proj_items(*pending_norm)
                fpos = 0

                den_all = ph2.tile([NH, QW], F32, tag="den", bufs=2)
                nkt = 4 * (qt + 1)
                nprs = nkt // 2
                # spread filler across all of this row's (pair, pr) iterations
                quota = -(-len(filler) // max(4 * nprs, 1))

                for hp in range(NH // 2):
                    dg = hp
                    hA, hB = 2 * hp, 2 * hp + 1
                    qA = qT[0:64, dg, qt * QW : (qt + 1) * QW]
                    qB = qT[64:128, dg, qt * QW : (qt + 1) * QW]
                    y_psA = psum_y.tile([D + 1, QW], F32, tag="yps")
                    y_psB = psum_y.tile([D + 1, QW], F32, tag="yps")

                    def emit_y(pending_pr, ptA, ptB):
                        for hf in range(2):
                            kt = 2 * pending_pr + hf
                            off = max(KW * kt - QW * qt, 0)
                            for p_t, h, y_ps in (
                                (ptA, hA, y_psA),
                                (ptB, hB, y_psB),
                            ):
                                # diag k-tiles: p cols [0:off] are zero
                                nc.tensor.matmul(
                                    y_ps[:, off:QW],
                                    v_sb[:, kt, h * VW : (h + 1) * VW],
                                    p_t[:, hf * QW + off : (hf + 1) * QW],
                                    start=(kt == 0),
                                    stop=(kt == nkt - 1),
                                    skip_group_check=True,
                                )

                    pending = None
                    for pr in range(nprs):
                        spA = ph2ps.tile([KW, 2 * QW], F32, tag="sps")
                        spB = ph2ps.tile([KW, 2 * QW], F32, tag="sps")
                        for hf in range(2):
                            kt = 2 * pr + hf
                            ksl = slice(kt * KW, (kt + 1) * KW)
                            off = KW * kt - QW * qt
                            for sp, lo, qfull in ((spA, 0, qA), (spB, 64, qB)):
                                scol = slice(hf * QW, (hf + 1) * QW)
                                if off < 256:
                                    nc.tensor.matmul(
                                        sp[:, scol],
                                        kT[lo : lo + 64, dg, ksl],
                                        qfull,
                                        start=True,
                                        stop=(off < 0),
                                    )
                                    if off >= 0:
                                        mw = off + KW  # masked col extent
                                        nc.tensor.matmul(
                                            sp[:, hf * QW : hf * QW + mw],
                                            eye_sb[:],
                                            mask_sb[:, 384 - off : 384 - off + mw],
                                            start=False,
                                            stop=True,
                                            skip_group_check=True,
                                        )
                                else:
                                    # cols [0:off] fully masked: write the
                                    # full mask, then add s on the rest
                                    nc.tensor.matmul(
                                        sp[:, scol],
                                        eye_sb[:],
                                        mask_sb[:, 384 - off : 896 - off],
                                        start=True,
                                        stop=False,
                                    )
                                    nc.tensor.matmul(
                                        sp[:, hf * QW + off : (hf + 1) * QW],
                                        kT[lo : lo + 64, dg, ksl],
                                        qT[
                                            lo : lo + 64,
                                            dg,
                                            qt * QW + off : (qt + 1) * QW,
                                        ],
                                        start=False,
                                        stop=True,
                                        skip_group_check=True,
                                    )
                        ptA = ph2.tile([KW, 2 * QW], BF16, tag="pt", bufs=4)
                        nc.scalar.activation(ptA[:], spA[:], EXP, scale=0.125)
                        ptB = ph2.tile([KW, 2 * QW], BF16, tag="pt", bufs=4)
                        nc.scalar.activation(ptB[:], spB[:], EXP, scale=0.125)
                        # software pipeline: y matmuls lag one iteration
                        if pending is not None:
                            emit_y(*pending)
                        pending = (pr, ptA, ptB)
                        # filler: QKV slabs + prev row norm/proj keep the PE
                        # fed while ScalarE runs the exps
                        for it in filler[fpos : fpos + quota]:
                            it()
                        fpos += quota
                    emit_y(*pending)
                    # spill unnormalized y into yT (freeing the psum slots);
                    # normalization is applied in-place one row later
                    for h, y_ps in ((hA, y_psA), (hB, y_psB)):
                        po = 64 * (h % 2)
                        ysl = yT[po : po + 64, dg, qt * QW : (qt + 1) * QW]
                        nc.vector.tensor_copy(ysl, y_ps[0:D, :])
                        den_st = ph2.tile([1, QW], F32, tag="denst", bufs=2)
                        nc.vector.tensor_copy(den_st[:], y_ps[D : D + 1, :])
                        # Pool-queue DMA: cheap issue, writes partition h
                        nc.gpsimd.dma_start(den_all[h : h + 1, :], den_st[:])

                for it in filler[fpos:]:
                    it()
                pending_norm = (qt, den_all)

            # ---------- epilogue: normalize + project last row ------------
            for it in norm_proj_items(*pending_norm):
                it()
    return nc


_CACHE = {}
last_exec_time_ns = None


def _causal_mask_np():
    # additive bias: 0 where visible (m >= tk + 384), -480 where masked
    m = np.full((KW, 896), -480.0, dtype=np.float32)
    tk = np.arange(KW)[:, None]
    mm = np.arange(896)[None, :]
    m[mm >= tk + 384] = 0.0
    return m.astype(BF16NP)


def kernel(x, w_attn, b_attn, w_proj, b_proj):
    global last_exec_time_ns
    x = np.asarray(x, dtype=np.float32)
    w_attn = np.asarray(w_attn, dtype=np.float32)
    b_attn = np.asarray(b_attn, dtype=np.float32)
    w_proj = np.asarray(w_proj, dtype=np.float32)
    b_proj = np.asarray(b_proj, dtype=np.float32)

    if "nc" not in _CACHE:
        _CACHE["nc"] = _build_program()
    nc = _CACHE["nc"]

    mask = _causal_mask_np()
    eye = np.eye(KW, dtype=np.float32).astype(BF16NP)
    sel = np.kron(
        np.eye(NH, dtype=np.float32), np.ones((1, D), np.float32)
    ).astype(BF16NP)
    in_maps = []
    for c in range(8):
        b, g = divmod(c, 2)
        s = slice(g * 512, (g + 1) * 512)
        in_maps.append(
            {
                "xT": np.ascontiguousarray(x[b].T).astype(BF16NP),
                "wq": np.ascontiguousarray(w_attn[:, s]).astype(BF16NP),
                "wk": np.ascontiguousarray(
                    w_attn[:, 1024 + g * 512 : 1024 + (g + 1) * 512]
                ).astype(BF16NP),
                "wv": np.ascontiguousarray(
                    w_attn[:, 2048 + g * 512 : 2048 + (g + 1) * 512]
                ).astype(BF16NP),
                "bq": np.ascontiguousarray(b_attn[s]),
                "bk": np.ascontiguousarray(b_attn[1024 + g * 512 : 1024 + (g + 1) * 512]),
                "wp": np.ascontiguousarray(w_proj[s, :]).astype(BF16NP),
                "mask": mask,
                "eye": eye,
                "sel": sel,
            }
        )

    res = run_bass_kernel_spmd(nc, in_maps, core_ids=list(range(8)))
    last_exec_time_ns = res.exec_time_ns

    # v-bias folds through softmax into a constant output bias (exact):
    bias_full = b_proj + b_attn[2048:].astype(np.float64) @ w_proj.astype(np.float64)
    bias_full = bias_full.astype(np.float32)

    out = np.empty((B, T, C), dtype=np.float32)
    for b in range(B):
        out[b] = (
            res.results[2 * b]["out"].astype(np.float32)
            + res.results[2 * b + 1]["out"].astype(np.float32)
            + bias_full
        )
    return out


# revision 32
# speedup vs baseline: 1.2729x; 1.0044x over previous
"""Causal self-attention Trainium2 kernel (8 NeuronCores, SPMD).

Problem: B=4, T=2048, C=1024, H=16 heads, D=64.
  qkv = x @ w_attn + b_attn ; causal softmax attention ; out = y @ w_proj + b_proj

Sharding: core c = 2*b + g  handles batch b with head-group g (heads 8g..8g+7).
Each core computes a partial projection output (its 8 heads' contribution);
the host sums the two partials per batch and adds b_proj + b_v @ w_proj
(the v-bias commutes through softmax normalization, so it folds into a
constant output bias added host-side).

All matmul operands are bf16 (psum accumulation stays fp32).

The kernel is software-pipelined at two levels to keep the PE dense (the
TRN2 PE p-state throttles after idle gaps, so PE must never starve):
  - within an attention row: y matmuls lag the exp by one iteration
  - across phases: QKV slab projections and the previous row's
    normalize+project work are interleaved between the attention
    iterations, filling the PE while ScalarE computes exp.

Row qt (q tokens [512qt, 512qt+512)) needs k/v tokens < 512(qt+1) =
slabs 0..2qt+1, so row qt runs while slabs 2qt+2, 2qt+3 are projected.

Causal handling per k-tile, offset off = 128*kt - 512*qt:
  off < 0   : plain matmul
  off 0/128 : matmul + eye@mask accumulate on first off+128 cols only
  off >= 256: eye@mask writes psum first (start=True), then the s-matmul
              adds only cols [off:512]  (cols [0:off] fully masked)
y matmuls on diag k-tiles touch only cols [off:512] (p is 0 below).
"""

import sys
import os

sys.path.insert(0, "/opt/trn_rl_repo")

import numpy as np
import concourse.bass as bass
import concourse.mybir as mybir
import concourse.tile as tile
from concourse.vector_clock import ScopedClock
from concourse.bass_utils import run_bass_kernel_spmd

F32 = mybir.dt.float32
BF16 = mybir.dt.bfloat16
EXP = mybir.ActivationFunctionType.Exp

B, T, C, H = 4, 2048, 1024, 16
D = C // H            # 64
NH = 8                # local heads per core
DG = 4                # d-groups of 128 partitions (2 heads each)
CK = 8                # contraction chunks of 128 over C
NQ = 4                # q tiles of 512
NT = 16               # t tiles of 128
QW = 512              # q tile width
KW = 128              # k tile width (partition dim of sT)
NR = 8                # ones columns per head: den replicated on 8 psum rows
VW = D + NR           # v block incl ones columns
SW = 256              # x slab width
NS = T // SW          # 8 slabs

BF16NP = mybir.dt.np(BF16)


# ---------------------------------------------------------------------------
# Tile compatibility patches for this walrus build: it accepts at most ONE
# sync wait per instruction, while TileContext attaches several.  Split the
# extras onto dedicated nops (same engine, just before the instruction).
# ---------------------------------------------------------------------------
def _install_patches():
    if getattr(tile.TileContext, "_wsplit_patched", False):
        return

    def _drain_and_barrier(self, tick_clock, wait_clock):
        drain_inst = self.nc.sync.drain()
        wait_clock.add_sem_waits(
            drain_inst.ins, ScopedClock({None: tick_clock.global_clock})
        )
        si = drain_inst.ins.sync_info
        waits = list(si.on_wait or []) if si is not None else []
        if len(waits) > 1:
            si.on_wait = waits[:1]
            for w in waits[1:]:
                n = self.nc.sync.nop(nofuse=True, hint="tail_wait")
                if n.ins.sync_info is None:
                    n.ins.sync_info = mybir.SyncInfo(on_wait=[w], on_update=[])
                else:
                    n.ins.sync_info.on_wait = [w]
        self.nc.all_engine_barrier()
        popped = self.nc._tile_sem_poison_stack.pop()
        assert popped is self._sem_poison
        self.nc.clear_and_free_semaphores(list(self.sems.allocated().values()))
        self.nc.all_engine_barrier()

    _orig_commit = tile.TileContext._commit_and_lower

    def _commit_and_lower(self, inst, original_block, old_bb_map, bb_to_exit_bb):
        si = getattr(inst, "sync_info", None)
        if si is not None and si.on_wait and len(si.on_wait) > 1:
            waits = list(si.on_wait)
            si.on_wait = [waits[-1]]
            for w in waits[:-1]:
                nop = self.nc.engines[inst.engine].nop(nofuse=True, hint="wsplit")
                if nop.ins.sync_info is None:
                    nop.ins.sync_info = mybir.SyncInfo(on_wait=[w], on_update=[])
                else:
                    nop.ins.sync_info.on_wait = [w]
        return _orig_commit(self, inst, original_block, old_bb_map, bb_to_exit_bb)

    tile.TileContext._drain_and_barrier = _drain_and_barrier
    tile.TileContext._commit_and_lower = _commit_and_lower
    tile.TileContext._wsplit_patched = True


# ---------------------------------------------------------------------------
# Kernel program
# ---------------------------------------------------------------------------
def _build_program():
    _install_patches()
    nc = bass.Bass()

    xT_e = nc.dram_tensor("xT", [C, T], BF16, kind="ExternalInput")
    wq_e = nc.dram_tensor("wq", [C, NH * D], BF16, kind="ExternalInput")
    wk_e = nc.dram_tensor("wk", [C, NH * D], BF16, kind="ExternalInput")
    wv_e = nc.dram_tensor("wv", [C, NH * D], BF16, kind="ExternalInput")
    bq_e = nc.dram_tensor("bq", [NH * D], F32, kind="ExternalInput")
    bk_e = nc.dram_tensor("bk", [NH * D], F32, kind="ExternalInput")
    wp_e = nc.dram_tensor("wp", [NH * D, C], BF16, kind="ExternalInput")
    mask_e = nc.dram_tensor("mask", [KW, 896], BF16, kind="ExternalInput")
    eye_e = nc.dram_tensor("eye", [KW, KW], BF16, kind="ExternalInput")
    sel_e = nc.dram_tensor("sel", [64, NH * D], BF16, kind="ExternalInput")
    out_e = nc.dram_tensor("out", [T, C], BF16, kind="ExternalOutput")

    with tile.TileContext(nc) as tc:
        with (
            tc.tile_pool(name="persist", bufs=1) as persist,
            tc.tile_pool(name="ph1x", bufs=2) as ph1x,
            tc.tile_pool(name="ph2", bufs=3) as ph2,
            tc.tile_pool(name="work", bufs=2) as work,
            tc.tile_pool(name="ph2ps", bufs=2, space="PSUM") as ph2ps,
            tc.tile_pool(name="psum_y", bufs=2, space="PSUM") as psum_y,
            tc.tile_pool(name="opsps", bufs=2, space="PSUM") as opsps,
        ):
            mask_sb = persist.tile([KW, 896], BF16, tag="mask")
            nc.sync.dma_start(mask_sb[:], mask_e[:])
            eye_sb = persist.tile([KW, KW], BF16, tag="eye")
            nc.sync.dma_start(eye_sb[:], eye_e[:])
            # selector: bc matmul picks head h's replicated recip rows
            # sel[p, h*64+i] = 1 iff p == 8h + (i % 8)
            sel_sb = persist.tile([64, NH * D], BF16, tag="sel")
            nc.sync.dma_start(sel_sb[:], sel_e[:])
            # q/k biases in column layout [d_in_group, dg] for the fused
            # bias-add on the DVE psum->sbuf spill (per-partition bias).
            bqk_sb = persist.tile([128, 2 * DG], F32, tag="bqk")
            nc.sync.dma_start(
                bqk_sb[:, 0:DG], bq_e.rearrange("(dg p) -> p dg", p=128)
            )
            nc.sync.dma_start(
                bqk_sb[:, DG : 2 * DG], bk_e.rearrange("(dg p) -> p dg", p=128)
            )

            wq_sb = persist.tile([128, CK, NH * D], BF16, tag="wq")
            wk_sb = persist.tile([128, CK, NH * D], BF16, tag="wk")
            wv_sb = persist.tile([128, CK, NH * D], BF16, tag="wv")
            wp_sb = persist.tile([128, DG, C], BF16, tag="wp")

            qT = persist.tile([128, DG, T], BF16, tag="qT")
            kT = persist.tile([128, DG, T], BF16, tag="kT")
            v_sb = persist.tile([128, NT, NH * VW], BF16, tag="v")
            yT = persist.tile([128, DG, T], BF16, tag="yT")
            # ones column (softmax denominator source), written once
            nc.vector.memset(
                v_sb[:].rearrange("p t (h e) -> p t h e", e=VW)[:, :, :, D : D + NR],
                1.0,
            )

            # ---------- work items: QKV slab pieces -----------------------
            xt_tiles = {}

            def slab_items(th):
                def dma_item():
                    xt = ph1x.tile([128, CK, SW], BF16, tag="xt")
                    xt_tiles[th] = xt
                    nc.sync.dma_start(
                        xt[:],
                        xT_e[:, th * SW : (th + 1) * SW].rearrange(
                            "(ck p) t -> p ck t", p=128
                        ),
                    )

                items = [dma_item]

                def qk_item(dst, w_sb, bc0, dg):
                    def run():
                        xt = xt_tiles[th]
                        ps = opsps.tile([KW, QW], F32, tag="ops")
                        for ck in range(CK):
                            nc.tensor.matmul(
                                ps[:, 0:SW],
                                w_sb[:, ck, dg * 128 : (dg + 1) * 128],
                                xt[:, ck, :],
                                start=(ck == 0),
                                stop=(ck == CK - 1),
                            )
                        nc.vector.tensor_scalar_add(
                            dst[:, dg, th * SW : th * SW + SW],
                            ps[:, 0:SW],
                            bqk_sb[:, bc0 + dg : bc0 + dg + 1],
                        )

                    return run

                for dst, w_sb, bc0 in ((qT, wq_sb, 0), (kT, wk_sb, DG)):
                    for dg in range(DG):
                        items.append(qk_item(dst, w_sb, bc0, dg))

                def v_item(tt):
                    def run():
                        xt = xt_tiles[th]
                        tta = th * (SW // 128) + tt
                        ps = opsps.tile([KW, QW], F32, tag="ops")
                        for ck in range(CK):
                            nc.tensor.matmul(
                                ps[:],
                                xt[:, ck, tt * 128 : (tt + 1) * 128],
                                wv_sb[:, ck, :],
                                start=(ck == 0),
                                stop=(ck == CK - 1),
                            )
                        v_blk = v_sb[:, tta, :].rearrange("p (h e) -> p h e", e=VW)
                        nc.vector.tensor_copy(
                            v_blk[:, :, 0:D],
                            ps[:].rearrange("p (h d) -> p h d", d=D),
                        )

                    return run

                for tt in range(SW // 128):
                    items.append(v_item(tt))
                return items

            # ---------- work items: normalize + project row nq ------------
            def recip_item_fn(recip_all, den_all, half):
                # half: None = all 64 rows, 0 = rows 0:32, 1 = rows 32:64
                sl = slice(None) if half is None else slice(32 * half, 32 * half + 32)

                def run():
                    with nc.allow_low_precision(reason="softmax reciprocal"):
                        nc.vector.reciprocal(recip_all[sl, :], den_all[sl, :])

                return run

            def norm_item_fn(nq, recip_all, h):
                def run():
                    dg = h // 2
                    po = 64 * (h % 2)
                    # broadcast recip rows 8h..8h+8 across 64 partitions on
                    # the PE (engines can't address base partition 8h); only
                    # contract over the 32-row half containing head h so the
                    # other half may be uninitialized (split-recip epilogue)
                    sl = slice(32 * (h // 4), 32 * (h // 4) + 32)
                    bc_slot = opsps.tile([KW, QW], F32, tag="ops")
                    bc_ps = bc_slot[0:D, 0:QW]
                    nc.tensor.matmul(
                        bc_ps,
                        sel_sb[sl, h * D : (h + 1) * D],
                        recip_all[sl, :],
                        start=True,
                        stop=True,
                    )
                    ysl = yT[po : po + 64, dg, nq * QW : (nq + 1) * QW]
                    nc.vector.tensor_mul(ysl, ysl, bc_ps)

                return run

            def norm_proj_items(nq, den_all):
                recip_all = ph2.tile([64, QW], BF16, tag="recip", bufs=2)
                items = [recip_item_fn(recip_all, den_all, None)]
                for h in range(NH):
                    items.append(norm_item_fn(nq, recip_all, h))
                items.extend(proj_items(nq))
                return items

            def proj_items(nq):
                def proj_item(tt, eh):
                    def run():
                        o_ps = opsps.tile([KW, QW], F32, tag="ops")
                        for jc in range(DG):
                            nc.tensor.matmul(
                                o_ps[:],
                                yT[:, jc, tt * 128 : (tt + 1) * 128],
                                wp_sb[:, jc, eh * QW : (eh + 1) * QW],
                                start=(jc == 0),
                                stop=(jc == DG - 1),
                            )
                        o_sb = work.tile([128, QW], BF16, tag="osb")
                        nc.vector.tensor_copy(o_sb[:], o_ps[:])
                        nc.sync.dma_start(
                            out_e[
                                tt * 128 : (tt + 1) * 128, eh * QW : (eh + 1) * QW
                            ],
                            o_sb[:],
                        )

                    return run

                return [
                    proj_item(tt, eh)
                    for tt in range(4 * nq, 4 * (nq + 1))
                    for eh in range(2)
                ]

            # ---------- prologue: slabs 0,1 -------------------------------
            # DMA order front-loads what the first matmuls need: wq + slab 0
            s0, s1 = slab_items(0), slab_items(1)
            nc.sync.dma_start(
                wq_sb[:], wq_e.rearrange("(ck p) d -> p ck d", p=128)
            )
            s0[0]()  # xt slab 0 DMA
            for w_sb, w_ext in ((wk_sb, wk_e), (wv_sb, wv_e)):
                nc.sync.dma_start(
                    w_sb[:], w_ext.rearrange("(ck p) d -> p ck d", p=128)
                )
            s1[0]()  # xt slab 1 DMA
            nc.sync.dma_start(
                wp_sb[:], wp_e.rearrange("(jc p) e -> p jc e", p=128)
            )
            for it in s0[1:] + s1[1:]:
                it()

            # ---------- attention rows with interleaved filler ------------
            pending_norm = None  # (nq, den_all) of previous row
            for qt in range(NQ):
                filler = []
                if qt < NQ - 1:
                    filler += slab_items(2 * qt + 2) + slab_items(2 * qt + 3)
                if pending_norm is not None:
                    filler += norm_proj_items(*pending_norm)
                fpos = 0

                den_all = ph2.tile([64, QW], F32, tag="den", bufs=2)
                nkt = 4 * (qt + 1)
                nprs = nkt // 2
                # spread filler across all of this row's (pair, pr) iterations
                quota = -(-len(filler) // max(4 * nprs, 1))
                # the last row normalizes its first half mid-row so only the
                # output projection is left for the epilogue
                last_recip = (
                    ph2.tile(
                        [64, QW], BF16, tag="recip", bufs=2, name="last_recip"
                    )
                    if qt == NQ - 1
                    else None
                )

                for hp in range(NH // 2):
                    dg = hp
                    hA, hB = 2 * hp, 2 * hp + 1
                    qA = qT[0:64, dg, qt * QW : (qt + 1) * QW]
                    qB = qT[64:128, dg, qt * QW : (qt + 1) * QW]
                    y_psA = psum_y.tile([D + NR, QW], F32, tag="yps")
                    y_psB = psum_y.tile([D + NR, QW], F32, tag="yps")

                    def emit_y(pending_pr, ptA, ptB):
                        for hf in range(2):
                            kt = 2 * pending_pr + hf
                            off = max(KW * kt - QW * qt, 0)
                            for p_t, h, y_ps in (
                                (ptA, hA, y_psA),
                                (ptB, hB, y_psB),
                            ):
                                # diag k-tiles: p cols [0:off] are zero
                                nc.tensor.matmul(
                                    y_ps[:, off:QW],
                                    v_sb[:, kt, h * VW : (h + 1) * VW],
                                    p_t[:, hf * QW + off : (hf + 1) * QW],
                                    start=(kt == 0),
                                    stop=(kt == nkt - 1),
                                    skip_group_check=True,
                                )

                    pending = None
                    for pr in range(nprs):
                        spA = ph2ps.tile([KW, 2 * QW], F32, tag="sps")
                        spB = ph2ps.tile([KW, 2 * QW], F32, tag="sps")
                        for hf in range(2):
                            kt = 2 * pr + hf
                            ksl = slice(kt * KW, (kt + 1) * KW)
                            off = KW * kt - QW * qt
                            for sp, lo, qfull in ((spA, 0, qA), (spB, 64, qB)):
                                scol = slice(hf * QW, (hf + 1) * QW)
                                if off < 256:
                                    nc.tensor.matmul(
                                        sp[:, scol],
                                        kT[lo : lo + 64, dg, ksl],
                                        qfull,
                                        start=True,
                                        stop=(off < 0),
                                    )
                                    if off >= 0:
                                        mw = off + KW  # masked col extent
                                        nc.tensor.matmul(
                                            sp[:, hf * QW : hf * QW + mw],
                                            eye_sb[:],
                                            mask_sb[:, 384 - off : 384 - off + mw],
                                            start=False,
                                            stop=True,
                                            skip_group_check=True,
                                        )
                                else:
                                    # cols [0:off] fully masked: write the
                                    # full mask, then add s on the rest
                                    nc.tensor.matmul(
                                        sp[:, scol],
                                        eye_sb[:],
                                        mask_sb[:, 384 - off : 896 - off],
                                        start=True,
                                        stop=False,
                                    )
                                    nc.tensor.matmul(
                                        sp[:, hf * QW + off : (hf + 1) * QW],
                                        kT[lo : lo + 64, dg, ksl],
                                        qT[
                                            lo : lo + 64,
                                            dg,
                                            qt * QW + off : (qt + 1) * QW,
                                        ],
                                        start=False,
                                        stop=True,
                                        skip_group_check=True,
                                    )
                        ptA = ph2.tile([KW, 2 * QW], BF16, tag="pt", bufs=4)
                        ptB = ph2.tile([KW, 2 * QW], BF16, tag="pt", bufs=4)
                        if pr == nprs - 1:
                            # offs are 256/384: the masked cols of p are
                            # never read by the narrowed y matmuls -- skip
                            # their exp entirely
                            for sp, pt_ in ((spA, ptA), (spB, ptB)):
                                nc.scalar.activation(
                                    pt_[:, 256:QW], sp[:, 256:QW], EXP, scale=0.125
                                )
                                nc.scalar.activation(
                                    pt_[:, QW + 384 : 2 * QW],
                                    sp[:, QW + 384 : 2 * QW],
                                    EXP,
                                    scale=0.125,
                                )
                        else:
                            nc.scalar.activation(ptA[:], spA[:], EXP, scale=0.125)
                            nc.scalar.activation(ptB[:], spB[:], EXP, scale=0.125)
                        # software pipeline: y matmuls lag one iteration
                        if pending is not None:
                            emit_y(*pending)
                        pending = (pr, ptA, ptB)
                        # filler: QKV slabs + prev row norm/proj keep the PE
                        # fed while ScalarE runs the exps
                        for it in filler[fpos : fpos + quota]:
                            it()
                        fpos += quota
                    emit_y(*pending)
                    # spill unnormalized y into yT (freeing the psum slots);
                    # normalization is applied in-place one row later
                    for h, y_ps in ((hA, y_psA), (hB, y_psB)):
                        po = 64 * (h % 2)
                        ysl = yT[po : po + 64, dg, qt * QW : (qt + 1) * QW]
                        nc.vector.tensor_copy(ysl, y_ps[0:D, :])
                        # den is replicated on 8 psum rows (8 ones columns in
                        # v) so one DMA fills the 8 recip-source partitions
                        den_st = ph2.tile([NR, QW], F32, tag="denst", bufs=2)
                        nc.vector.tensor_copy(den_st[:], y_ps[D : D + NR, :])
                        # Pool-queue DMA: cheap issue, writes partition 8h
                        nc.gpsimd.dma_start(
                            den_all[NR * h : NR * (h + 1), :], den_st[:]
                        )

                    if qt == NQ - 1 and hp == 1:
                        # first half's dens are complete: normalize heads
                        # 0..3 while pairs 2,3 still run attention
                        filler.append(recip_item_fn(last_recip, den_all, 0))
                        for h in range(4):
                            filler.append(norm_item_fn(qt, last_recip, h))

                for it in filler[fpos:]:
                    it()
                pending_norm = (qt, den_all)

            # ---------- epilogue: finish last row norm, project it --------
            qt3 = NQ - 1
            recip_item_fn(last_recip, pending_norm[1], 1)()
            for h in range(4, NH):
                norm_item_fn(qt3, last_recip, h)()
            for it in proj_items(qt3):
                it()
    return nc


_CACHE = {}
last_exec_time_ns = None


def _causal_mask_np():
    # additive bias: 0 where visible (m >= tk + 384), -480 where masked
    m = np.full((KW, 896), -480.0, dtype=np.float32)
    tk = np.arange(KW)[:, None]
    mm = np.arange(896)[None, :]
    m[mm >= tk + 384] = 0.0
    return m.astype(BF16NP)


def kernel(x, w_attn, b_attn, w_proj, b_proj):
    global last_exec_time_ns
    x = np.asarray(x, dtype=np.float32)
    w_attn = np.asarray(w_attn, dtype=np.float32)
    b_attn = np.asarray(b_attn, dtype=np.float32)
    w_proj = np.asarray(w_proj, dtype=np.float32)
    b_proj = np.asarray(b_proj, dtype=np.float32)

    if "nc" not in _CACHE:
        _CACHE["nc"] = _build_program()
    nc = _CACHE["nc"]

    mask = _causal_mask_np()
    eye = np.eye(KW, dtype=np.float32).astype(BF16NP)
    # sel[p, h*64+i] = 1 iff p == 8h + (i % 8): picks head h's replicated
    # recip rows and spreads them across all 64 output partitions
    sel = np.zeros((64, NH * D), dtype=np.float32)
    for h in range(NH):
        for i in range(D):
            sel[NR * h + i % NR, h * D + i] = 1.0
    sel = sel.astype(BF16NP)
    in_maps = []
    for c in range(8):
        b, g = divmod(c, 2)
        s = slice(g * 512, (g + 1) * 512)
        in_maps.append(
            {
                "xT": np.ascontiguousarray(x[b].T).astype(BF16NP),
                "wq": np.ascontiguousarray(w_attn[:, s]).astype(BF16NP),
                "wk": np.ascontiguousarray(
                    w_attn[:, 1024 + g * 512 : 1024 + (g + 1) * 512]
                ).astype(BF16NP),
                "wv": np.ascontiguousarray(
                    w_attn[:, 2048 + g * 512 : 2048 + (g + 1) * 512]
                ).astype(BF16NP),
                "bq": np.ascontiguousarray(b_attn[s]),
                "bk": np.ascontiguousarray(b_attn[1024 + g * 512 : 1024 + (g + 1) * 512]),
                "wp": np.ascontiguousarray(w_proj[s, :]).astype(BF16NP),
                "mask": mask,
                "eye": eye,
                "sel": sel,
            }
        )

    res = run_bass_kernel_spmd(nc, in_maps, core_ids=list(range(8)))
    last_exec_time_ns = res.exec_time_ns

    # v-bias folds through softmax into a constant output bias (exact):
    bias_full = b_proj + b_attn[2048:].astype(np.float64) @ w_proj.astype(np.float64)
    bias_full = bias_full.astype(np.float32)

    out = np.empty((B, T, C), dtype=np.float32)
    for b in range(B):
        out[b] = (
            res.results[2 * b]["out"].astype(np.float32)
            + res.results[2 * b + 1]["out"].astype(np.float32)
            + bias_full
        )
    return out
